# revision 29
# baseline (speedup 1.0000x reference)
"""Trainium2 Bass kernel for nn_AverageAttention: cumulative-average attention
with a sigmoid gating Linear(2D->2D).

Strategy: data-parallel over batch (B=8 = one batch element per NeuronCore).
All on-chip work happens in transposed space ([feature, token]); the gating
matmul runs in FP8-E4M3 DoubleRow mode (2 contraction rows/cycle -> 2x the
bf16 PE throughput; ~437us PE floor vs 874us bf16):
  - host pre-quantizes x (x16) and W (x256 for the x-rows, x128 for the
    avg-rows) to fp8e4 clipped to +-240; the device casts the scan output
    avg to fp8 (x32) on ScalarE; all products carry a uniform 4096 scale
    that the sigmoid activation undoes via its scale operand (1/4096)
  - cumavg via the affine recurrence avg_t = coef_t*avg_{t-1} + x_t/(t+1),
    one fused tensor_tensor_scan per 512-col chunk on VectorE; slice-0
    scans run eagerly (pass-1 avg-halves consume them), slices 1-3 are
    threaded ~3 scans per unit through pass 1 and the start of pass 2 so
    the in-order vector/scalar streams never head-block epilogues
  - the combine (sigmoid*x + sigmoid*avg) runs all-bf16 on VectorE
    (2-byte operands engage the DVE 2x_1p mode); x streams in as bf16
    per unit, avg is the resident bf16 scan output; outputs stored bf16
  - queue discipline (a DMA consumer waits on its queue position):
    sync HWDGE carries ONLY W (runway x-rows first, pass-2 tiles
    prefetched one i-group ahead); scalar HWDGE carries coef/x-fp8/bias
    and the pass-1 x-bf16 stream; gpsimd SWDGE carries xd/avg/out and
    the pass-2 x-bf16 stream
  - every DRAM tensor is packed so DMA lines are contiguous: W and x-fp8
    8KB/partition, the streamed bf16 tiles 128KB contiguous blocks
"""
import sys

if "/opt/trn_rl_repo" not in sys.path:
    sys.path.insert(0, "/opt/trn_rl_repo")

import numpy as np
import ml_dtypes

B, T, D = 8, 2048, 2048
O = 2 * D          # gate output features (4096)
P = 128            # partitions
KT = D // P        # 16 k-tiles per half of G
DT = D // P        # 16 output-feature tiles
NK = 2 * KT        # 32 k-tiles total
NPR = KT // 2      # 8 DoubleRow pairs per half
TS = 512           # t-slice (matmul moving free dim / scan chunk)
NS = T // TS       # 4 t-slices
RUNWAY = 3         # units whose x-half matmuls front-run the scans

S_X, S_W1 = 16.0, 256.0    # x and W-x-rows fp8 scales
S_A, S_W2 = 32.0, 128.0    # avg and W-avg-rows fp8 scales
INV_S = 1.0 / (S_X * S_W1)  # = 1/(S_A*S_W2): undone inside the sigmoid

_compiled = None


def _build():
    import concourse.mybir as mybir
    import concourse.tile as tile
    from concourse import bacc

    f32 = mybir.dt.float32
    bf16 = mybir.dt.bfloat16
    f8 = mybir.dt.float8e4
    SIG = mybir.ActivationFunctionType.Sigmoid
    CPY = mybir.ActivationFunctionType.Copy
    DR = mybir.MatmulPerfMode.DoubleRow

    nc = bacc.Bacc(trn_type="TRN2", target_bir_lowering=False, debug=False,
                   num_devices=B)

    xf8_d = nc.declare_dram_parameter("xf8T", [P, NS, KT, TS], f8,
                                      isOutput=False)
    xe_d = nc.declare_dram_parameter("xeT", [NS, DT, P, TS], bf16,
                                     isOutput=False)
    xd_d = nc.declare_dram_parameter("xdT", [NS, KT, P, TS], bf16,
                                     isOutput=False)
    wP_d = nc.declare_dram_parameter("wP", [DT, P, NK * 2 * P], f8,
                                     isOutput=False)
    bias_d = nc.declare_dram_parameter("bias", [O], f32, isOutput=False)
    coef_d = nc.declare_dram_parameter("coef_t", [1, T], f32, isOutput=False)
    avg_d = nc.declare_dram_parameter("avgT", [NS, KT, P, TS], bf16,
                                      isOutput=True)
    out_d = nc.declare_dram_parameter("outT", [NS, DT, P, TS], bf16,
                                      isOutput=True)

    with tile.TileContext(nc) as tc:
        with tc.tile_pool(name="consts", bufs=1) as consts, \
             tc.tile_pool(name="resid", bufs=1) as resid, \
             tc.tile_pool(name="xdp", bufs=16) as xdp, \
             tc.tile_pool(name="xep", bufs=4) as xep, \
             tc.tile_pool(name="wpool", bufs=5) as wpool, \
             tc.tile_pool(name="sigp", bufs=4) as sigp, \
             tc.tile_pool(name="outp", bufs=4) as outp, \
             tc.tile_pool(name="psum", bufs=8, space="PSUM") as pp:

            # scalar HWDGE queue carries ONLY coef slice 0 and the pass-1
            # x-bf16 stream: any transfer backlog on the scalar queue makes
            # dma_start issue instructions block the scalar ENGINE stream,
            # which head-blocks the fp8 casts and sigmoids behind them.
            # Everything else (W, x-fp8, bias, coef rest) rides the sync
            # queue in exact need-order.
            coef_sb = consts.tile([P, T], f32)
            nc.scalar.dma_start(out=coef_sb[:, 0:TS],
                                in_=coef_d[:, 0:TS].to_broadcast((P, TS)))
            bias0_sb = consts.tile([P, O // P], f32)
            nc.scalar.dma_start(
                out=bias0_sb, in_=bias_d.rearrange("(c p) -> p c", p=P))
            nc.scalar.dma_start(out=coef_sb[:, TS:T],
                                in_=coef_d[:, TS:T].to_broadcast((P, T - TS)))

            xf8 = resid.tile([P, KT, T], f8)
            avgT_bf = resid.tile([P, KT, T], bf16)
            af8 = resid.tile([P, KT, T], f8)
            bias_sb = bias0_sb
            carry = consts.tile([P, KT], f32)

            def load_xf8_slice(s, split=1):
                sl = slice(s * TS, (s + 1) * TS)
                step = KT // split
                for c in range(split):
                    js = slice(c * step, (c + 1) * step)
                    nc.sync.dma_start(out=xf8[:, js, sl],
                                      in_=xf8_d[:, s, js, :])

            def w_src(i):
                return wP_d[i].rearrange("p (kt c) -> p kt c", c=2 * P)

            def load_w_x(i, split=1):
                w_i = wpool.tile([P, NK, 2 * P], f8, tag="w")
                step = KT // split
                for c in range(split):
                    ks = slice(c * step, (c + 1) * step)
                    nc.sync.dma_start(out=w_i[:, ks, :], in_=w_src(i)[:, ks, :])
                return w_i

            def load_w_avg(w_i, i, eng=None):
                (eng or nc.sync).dma_start(out=w_i[:, KT:NK, :],
                                           in_=w_src(i)[:, KT:NK, :])

            def load_w(i):
                w_i = wpool.tile([P, NK, 2 * P], f8, tag="w")
                nc.sync.dma_start(out=w_i[:, :, :], in_=w_src(i))
                return w_i

            # ---- scan machinery: (s, j) work items. Slice 0 is eager;
            # ---- slices 1-3 are threaded a few at a time between units.
            def scan_one(s, j, xd):
                sl = slice(s * TS, (s + 1) * TS)
                avc = avgT_bf[:, j, sl]
                nc.vector.tensor_tensor_scan(
                    out=avc, data0=coef_sb[:, sl], data1=xd,
                    initial=(0.0 if s == 0 else carry[:, j:j + 1]),
                    op0=mybir.AluOpType.mult, op1=mybir.AluOpType.add)
                if s < NS - 1:
                    nc.vector.tensor_copy(carry[:, j:j + 1],
                                          avc[:, TS - 1:TS])
                nc.scalar.activation(af8[:, j, sl], avc, CPY, scale=S_A)
                deferred_avg.append((s, j))

            pend = [(s, j) for s in range(1, NS) for j in range(KT)]
            xd_tiles = {}
            state = {"xd": 0, "scan": 0}
            # avg-output stores are deferred into pass 2: the writes would
            # otherwise compete with the W/x/xd reads in the congested
            # pass-1 HBM window (the scan data stays resident in SBUF)
            deferred_avg = []

            def emit_avg_stores(n):
                for _ in range(min(n, len(deferred_avg))):
                    s, j = deferred_avg.pop(0)
                    nc.gpsimd.dma_start(out=avg_d[s, j],
                                        in_=avgT_bf[:, j,
                                                    s * TS:(s + 1) * TS])

            def issue_xd(n):
                hi = min(state["xd"] + n, len(pend))
                for k in range(state["xd"], hi):
                    s, j = pend[k]
                    xd = xdp.tile([P, TS], bf16, tag="xd")
                    nc.gpsimd.dma_start(out=xd, in_=xd_d[s, j])
                    xd_tiles[(s, j)] = xd
                state["xd"] = hi

            def emit_scans(n):
                hi = min(state["scan"] + n, len(pend))
                for k in range(state["scan"], hi):
                    s, j = pend[k]
                    scan_one(s, j, xd_tiles.pop((s, j)))
                state["scan"] = hi

            def scan_set0(inject=None):
                xds = []
                for j in range(KT):
                    if inject and j in inject:
                        inject[j]()
                    xd = xdp.tile([P, TS], bf16, tag="xd")
                    nc.gpsimd.dma_start(out=xd, in_=xd_d[0, j])
                    xds.append(xd)
                for j in range(KT):
                    scan_one(0, j, xds[j])

            def mm_halves(ps_ig, ps_fg, w_i, s, half):
                sl = slice(s * TS, (s + 1) * TS)
                base = 0 if half == 0 else KT
                rhs_t = xf8 if half == 0 else af8
                for q in range(NPR):
                    kk = slice(base + 2 * q, base + 2 * q + 2)
                    rr = slice(2 * q, 2 * q + 2)
                    nc.tensor.matmul(ps_ig, lhsT=w_i[:, kk, 0:P],
                                     rhs=rhs_t[:, rr, sl],
                                     start=(half == 0 and q == 0),
                                     stop=(half == 1 and q == NPR - 1),
                                     perf_mode=DR)
                for q in range(NPR):
                    kk = slice(base + 2 * q, base + 2 * q + 2)
                    rr = slice(2 * q, 2 * q + 2)
                    nc.tensor.matmul(ps_fg, lhsT=w_i[:, kk, P:2 * P],
                                     rhs=rhs_t[:, rr, sl],
                                     start=(half == 0 and q == 0),
                                     stop=(half == 1 and q == NPR - 1),
                                     perf_mode=DR)

            def epilogue(ps_ig, ps_fg, i, s, xe):
                """bf16 sigmoid outputs + all-bf16 combine: every DVE
                operand is 2-byte so the TT ops run in 2x_1p mode."""
                sl = slice(s * TS, (s + 1) * TS)
                sig_i = sigp.tile([P, TS], bf16, tag="sig")
                nc.scalar.activation(sig_i, ps_ig, SIG,
                                     bias=bias_sb[:, i:i + 1], scale=INV_S)
                sig_f = sigp.tile([P, TS], bf16, tag="sig")
                nc.scalar.activation(sig_f, ps_fg, SIG,
                                     bias=bias_sb[:, KT + i:KT + i + 1],
                                     scale=INV_S)
                nc.vector.tensor_mul(sig_i, sig_i, xe)
                nc.vector.tensor_mul(sig_f, sig_f, avgT_bf[:, i, sl])
                out_s = outp.tile([P, TS], bf16, tag="out")
                nc.vector.tensor_add(out_s, sig_i, sig_f)
                nc.gpsimd.dma_start(out=out_d[s, i], in_=out_s)

            def load_xe(i, s, eng):
                xe = xep.tile([P, TS], bf16, tag="xe")
                eng.dma_start(out=xe, in_=xe_d[s, i])
                return xe

            def full_unit(w_i, i, s, xe_eng):
                xe = load_xe(i, s, xe_eng)
                ps_ig = pp.tile([P, TS], f32, tag="ps")
                ps_fg = pp.tile([P, TS], f32, tag="ps")
                mm_halves(ps_ig, ps_fg, w_i, s, half=0)
                mm_halves(ps_ig, ps_fg, w_i, s, half=1)
                epilogue(ps_ig, ps_fg, i, s, xe)

            # ---- pass 1 (s = 0 across all i) ----
            run_xe = [load_xe(i, 0, nc.scalar) for i in range(RUNWAY)]
            # runway W avg-rows ride the gpsimd SWDGE queue, woven between
            # the slice-0 xd issues: the sync queue's serial transfer chain
            # (w0/xf8/w1x/w2x) is the startup bottleneck and these 1.5MB
            # would otherwise trail it by ~10us
            run_w = []
            for _ri in range(RUNWAY):
                w_tile = wpool.tile([P, NK, 2 * P], f8, tag="w")
                run_w.append(w_tile)
            scan_set0(inject={
                0: lambda: load_w_avg(run_w[0], 0, nc.gpsimd),
                5: lambda: load_w_avg(run_w[1], 1, nc.gpsimd),
                9: lambda: load_w_avg(run_w[2], 2, nc.gpsimd),
            })
            run_ps = []
            for i in range(RUNWAY):
                w_i = run_w[i]
                if i == 0:
                    # few, large transfers in strict need-order: each queue
                    # transfer carries ~1us fixed latency, so the first
                    # matmul's inputs (w0 pair-0 rows + xf8 j0-7) lead.
                    nc.sync.dma_start(out=w_i[:, 0:2, :],
                                      in_=w_src(0)[:, 0:2, :])
                    nc.sync.dma_start(out=xf8[:, 0:8, 0:TS],
                                      in_=xf8_d[:, 0, 0:8, :])
                    nc.sync.dma_start(out=w_i[:, 2:KT, :],
                                      in_=w_src(0)[:, 2:KT, :])
                    nc.sync.dma_start(out=xf8[:, 8:KT, 0:TS],
                                      in_=xf8_d[:, 0, 8:KT, :])
                else:
                    nc.sync.dma_start(out=w_i[:, 0:KT, :],
                                      in_=w_src(i)[:, 0:KT, :])
                ps_ig = pp.tile([P, TS], f32, tag="ps")
                ps_fg = pp.tile([P, TS], f32, tag="ps")
                mm_halves(ps_ig, ps_fg, w_i, 0, half=0)
                run_ps.append((w_i, ps_ig, ps_fg))
            for i in range(RUNWAY):
                w_i, ps_ig, ps_fg = run_ps[i]
                mm_halves(ps_ig, ps_fg, w_i, 0, half=1)
                epilogue(ps_ig, ps_fg, i, 0, run_xe[i])

            w_cur = load_w(RUNWAY)
            p2w0 = None
            for i in range(RUNWAY, DT):
                if i + 1 < DT:
                    w_nxt = load_w(i + 1)
                else:
                    w_nxt = p2w0        # pass-2 w0, prefetched at i==13
                issue_xd(5 if i >= 12 else 4)
                emit_scans(4 if i >= 13 else 3)
                full_unit(w_cur, i, 0, nc.scalar)
                if i == 5:
                    load_xf8_slice(1)
                elif i == 8:
                    load_xf8_slice(2)
                elif i == 10:
                    load_xf8_slice(3)
                elif i == 14:
                    p2w0 = load_w(0)   # after w15's load: don't cut its line
                w_cur = w_nxt

            # ---- pass 2: remaining slices, i-outer, W prefetched one
            # ---- i-group ahead; leftover scans finish in the first
            # ---- pass-2 units (their vector stream is nearly idle, but
            # ---- skip the very first unit so its sigmoids aren't queued
            # ---- behind scan-gated casts on the scalar stream) ----
            first_p2 = True
            for i in range(DT):
                w_i = w_cur
                for s in range(1, NS):
                    xe = load_xe(i, s, nc.gpsimd)
                    if not first_p2 and state["scan"] < len(pend):
                        issue_xd(3)
                        emit_scans(3)
                    if not first_p2:
                        emit_avg_stores(2)
                    first_p2 = False
                    if s == 2 and i + 1 < DT:
                        w_cur = load_w(i + 1)
                    ps_ig = pp.tile([P, TS], f32, tag="ps")
                    ps_fg = pp.tile([P, TS], f32, tag="ps")
                    mm_halves(ps_ig, ps_fg, w_i, s, half=0)
                    mm_halves(ps_ig, ps_fg, w_i, s, half=1)
                    epilogue(ps_ig, ps_fg, i, s, xe)

    nc.compile()
    return nc


def _get_compiled():
    global _compiled
    if _compiled is None:
        _compiled = _build()
    return _compiled


def _run(inputs, trace=False, **spmd_kwargs):
    from concourse.bass_utils import run_bass_kernel_spmd

    nc = _get_compiled()
    layer_in = np.asarray(inputs["layer_in"], dtype=np.float32)
    W_gate = np.asarray(inputs["W_gate"], dtype=np.float32)
    b_gate = np.asarray(inputs["b_gate"], dtype=np.float32)
    f8 = ml_dtypes.float8_e4m3
    bf = ml_dtypes.bfloat16

    # wP[i, p, kt*256+c]: W^T row kt*128+p, col (c>=128)*D + i*128 + (c%128),
    # x-rows (kt<16) scaled by S_W1, avg-rows by S_W2, fp8e4 clipped +-240.
    wT = np.ascontiguousarray(W_gate.T)                    # [k, o]
    w4 = wT.reshape(NK, P, 2, DT, P)
    scale = np.where(np.arange(NK) < KT, S_W1, S_W2).astype(np.float32)
    w5 = w4.transpose(3, 1, 0, 2, 4) * scale[None, None, :, None, None]
    wP = np.ascontiguousarray(
        np.clip(w5, -240, 240).astype(f8).reshape(DT, P, NK * 2 * P))

    tt = np.arange(T, dtype=np.float32)
    coef = (tt / (tt + 1.0)).reshape(1, T)
    inv = (1.0 / (tt + 1.0)).reshape(1, T)

    in_maps = []
    for bi in range(B):
        xT = np.ascontiguousarray(layer_in[bi].T)          # [D, T]
        xf8_h = (np.clip(xT * S_X, -240, 240).astype(f8)
                 .reshape(KT, P, NS, TS).transpose(1, 2, 0, 3))
        xe_h = (xT.astype(bf)
                .reshape(DT, P, NS, TS).transpose(2, 0, 1, 3))
        xd_h = ((xT * inv).astype(bf)
                .reshape(KT, P, NS, TS).transpose(2, 0, 1, 3))
        in_maps.append({
            "xf8T": np.ascontiguousarray(xf8_h),
            "xeT": np.ascontiguousarray(xe_h),
            "xdT": np.ascontiguousarray(xd_h),
            "wP": wP,
            "bias": b_gate,
            "coef_t": coef,
        })

    res = run_bass_kernel_spmd(nc, in_maps, core_ids=list(range(B)),
                               trace=trace, **spmd_kwargs)
    gating = np.empty((B, T, D), dtype=np.float32)
    avg = np.empty((B, T, D), dtype=np.float32)
    for bi in range(B):
        o4 = res.results[bi]["outT"].astype(np.float32)    # [NS, DT, P, TS]
        a4 = res.results[bi]["avgT"].astype(np.float32)
        gating[bi] = o4.transpose(1, 2, 0, 3).reshape(D, T).T
        avg[bi] = a4.transpose(1, 2, 0, 3).reshape(D, T).T
    return (gating, avg), res


def kernel(**inputs):
    (gating, avg), _ = _run(inputs, trace=False)
    return gating, avg


# revision 32
# speedup vs baseline: 1.0212x; 1.0212x over previous
"""Trainium2 Bass kernel for nn_AverageAttention: cumulative-average attention
with a sigmoid gating Linear(2D->2D).

Strategy: data-parallel over batch (B=8 = one batch element per NeuronCore).
All on-chip work happens in transposed space ([feature, token]); the gating
matmul runs in FP8-E4M3 DoubleRow mode (2 contraction rows/cycle -> 2x the
bf16 PE throughput; ~437us PE floor vs 874us bf16):
  - host pre-quantizes x (x16) and W (x256 for the x-rows, x128 for the
    avg-rows) to fp8e4 clipped to +-240; the device casts the scan output
    avg to fp8 (x32) on ScalarE; all products carry a uniform 4096 scale
    that the sigmoid activation undoes via its scale operand (1/4096)
  - cumavg via the affine recurrence avg_t = coef_t*avg_{t-1} + x_t/(t+1),
    one fused tensor_tensor_scan per 512-col chunk on VectorE; slice-0
    scans run eagerly (pass-1 avg-halves consume them), slices 1-3 are
    threaded ~3 scans per unit through pass 1 and the start of pass 2 so
    the in-order vector/scalar streams never head-block epilogues
  - the combine (sigmoid*x + sigmoid*avg) runs all-bf16 on VectorE
    (2-byte operands engage the DVE 2x_1p mode); x streams in as bf16
    per unit, avg is the resident bf16 scan output; outputs stored bf16
  - queue discipline (a DMA consumer waits on its queue position):
    sync HWDGE carries ONLY W (runway x-rows first, pass-2 tiles
    prefetched one i-group ahead); scalar HWDGE carries coef/x-fp8/bias
    and the pass-1 x-bf16 stream; gpsimd SWDGE carries xd/avg/out and
    the pass-2 x-bf16 stream
  - every DRAM tensor is packed so DMA lines are contiguous: W and x-fp8
    8KB/partition, the streamed bf16 tiles 128KB contiguous blocks
"""
import sys

if "/opt/trn_rl_repo" not in sys.path:
    sys.path.insert(0, "/opt/trn_rl_repo")

import numpy as np
import ml_dtypes

B, T, D = 8, 2048, 2048
O = 2 * D          # gate output features (4096)
P = 128            # partitions
KT = D // P        # 16 k-tiles per half of G
DT = D // P        # 16 output-feature tiles
NK = 2 * KT        # 32 k-tiles total
NPR = KT // 2      # 8 DoubleRow pairs per half
TS = 512           # t-slice (matmul moving free dim / scan chunk)
NS = T // TS       # 4 t-slices
RUNWAY = 3         # units whose x-half matmuls front-run the scans

S_X, S_W1 = 16.0, 256.0    # x and W-x-rows fp8 scales
S_A, S_W2 = 32.0, 128.0    # avg and W-avg-rows fp8 scales
INV_S = 1.0 / (S_X * S_W1)  # = 1/(S_A*S_W2): undone inside the sigmoid

_compiled = None


def _build():
    import concourse.mybir as mybir
    import concourse.tile as tile
    from concourse import bacc

    f32 = mybir.dt.float32
    bf16 = mybir.dt.bfloat16
    f8 = mybir.dt.float8e4
    SIG = mybir.ActivationFunctionType.Sigmoid
    CPY = mybir.ActivationFunctionType.Copy
    DR = mybir.MatmulPerfMode.DoubleRow

    nc = bacc.Bacc(trn_type="TRN2", target_bir_lowering=False, debug=False,
                   num_devices=B)

    xf8_d = nc.declare_dram_parameter("xf8T", [P, NS, KT, TS], f8,
                                      isOutput=False)
    xe_d = nc.declare_dram_parameter("xeT", [NS, DT, P, TS], bf16,
                                     isOutput=False)
    xd_d = nc.declare_dram_parameter("xdT", [NS, KT, P, TS], bf16,
                                     isOutput=False)
    wP_d = nc.declare_dram_parameter("wP", [DT, P, NK * 2 * P], f8,
                                     isOutput=False)
    bias_d = nc.declare_dram_parameter("bias", [O], f32, isOutput=False)
    coef_d = nc.declare_dram_parameter("coef_t", [1, T], f32, isOutput=False)
    avg_d = nc.declare_dram_parameter("avgT", [NS, KT, P, TS], bf16,
                                      isOutput=True)
    out_d = nc.declare_dram_parameter("outT", [NS, DT, P, TS], bf16,
                                      isOutput=True)

    with tile.TileContext(nc) as tc:
        with tc.tile_pool(name="consts", bufs=1) as consts, \
             tc.tile_pool(name="resid", bufs=1) as resid, \
             tc.tile_pool(name="xdp", bufs=16) as xdp, \
             tc.tile_pool(name="xep", bufs=4) as xep, \
             tc.tile_pool(name="wpool", bufs=5) as wpool, \
             tc.tile_pool(name="sigp", bufs=4) as sigp, \
             tc.tile_pool(name="outp", bufs=4) as outp, \
             tc.tile_pool(name="psum", bufs=8, space="PSUM") as pp:

            # scalar HWDGE queue carries ONLY coef slice 0 and the pass-1
            # x-bf16 stream: any transfer backlog on the scalar queue makes
            # dma_start issue instructions block the scalar ENGINE stream,
            # which head-blocks the fp8 casts and sigmoids behind them.
            # Everything else (W, x-fp8, bias, coef rest) rides the sync
            # queue in exact need-order.
            coef_sb = consts.tile([P, T], f32)
            nc.scalar.dma_start(out=coef_sb[:, 0:TS],
                                in_=coef_d[:, 0:TS].to_broadcast((P, TS)))
            bias0_sb = consts.tile([P, O // P], f32)
            nc.scalar.dma_start(
                out=bias0_sb, in_=bias_d.rearrange("(c p) -> p c", p=P))
            nc.scalar.dma_start(out=coef_sb[:, TS:T],
                                in_=coef_d[:, TS:T].to_broadcast((P, T - TS)))

            xf8 = resid.tile([P, KT, T], f8)
            avgT_bf = resid.tile([P, KT, T], bf16)
            af8 = resid.tile([P, KT, T], f8)
            bias_sb = bias0_sb
            carry = consts.tile([P, KT], f32)

            def load_xf8_slice(s, split=1):
                sl = slice(s * TS, (s + 1) * TS)
                step = KT // split
                for c in range(split):
                    js = slice(c * step, (c + 1) * step)
                    nc.sync.dma_start(out=xf8[:, js, sl],
                                      in_=xf8_d[:, s, js, :])

            def w_src(i):
                return wP_d[i].rearrange("p (kt c) -> p kt c", c=2 * P)

            def load_w_x(i, split=1):
                w_i = wpool.tile([P, NK, 2 * P], f8, tag="w")
                step = KT // split
                for c in range(split):
                    ks = slice(c * step, (c + 1) * step)
                    nc.sync.dma_start(out=w_i[:, ks, :], in_=w_src(i)[:, ks, :])
                return w_i

            def load_w_avg(w_i, i, eng=None):
                (eng or nc.sync).dma_start(out=w_i[:, KT:NK, :],
                                           in_=w_src(i)[:, KT:NK, :])

            def load_w(i):
                w_i = wpool.tile([P, NK, 2 * P], f8, tag="w")
                nc.sync.dma_start(out=w_i[:, :, :], in_=w_src(i))
                return w_i

            # ---- scan machinery: (s, j) work items. Slice 0 is eager;
            # ---- slices 1-3 are threaded a few at a time between units.
            def scan_one(s, j, xd):
                sl = slice(s * TS, (s + 1) * TS)
                avc = avgT_bf[:, j, sl]
                nc.vector.tensor_tensor_scan(
                    out=avc, data0=coef_sb[:, sl], data1=xd,
                    initial=(0.0 if s == 0 else carry[:, j:j + 1]),
                    op0=mybir.AluOpType.mult, op1=mybir.AluOpType.add)
                if s < NS - 1:
                    nc.vector.tensor_copy(carry[:, j:j + 1],
                                          avc[:, TS - 1:TS])
                nc.scalar.activation(af8[:, j, sl], avc, CPY, scale=S_A)
                nc.gpsimd.dma_start(out=avg_d[s, j], in_=avc)

            pend = [(s, j) for s in range(1, NS) for j in range(KT)]
            xd_tiles = {}
            state = {"xd": 0, "scan": 0}

            def issue_xd(n):
                hi = min(state["xd"] + n, len(pend))
                for k in range(state["xd"], hi):
                    s, j = pend[k]
                    xd = xdp.tile([P, TS], bf16, tag="xd")
                    nc.gpsimd.dma_start(out=xd, in_=xd_d[s, j])
                    xd_tiles[(s, j)] = xd
                state["xd"] = hi

            def emit_scans(n):
                hi = min(state["scan"] + n, len(pend))
                for k in range(state["scan"], hi):
                    s, j = pend[k]
                    scan_one(s, j, xd_tiles.pop((s, j)))
                state["scan"] = hi

            def scan_set0(inject=None):
                xds = []
                for j in range(KT):
                    if inject and j in inject:
                        inject[j]()
                    xd = xdp.tile([P, TS], bf16, tag="xd")
                    nc.gpsimd.dma_start(out=xd, in_=xd_d[0, j])
                    xds.append(xd)
                for j in range(KT):
                    scan_one(0, j, xds[j])

            def mm_halves(ps_ig, ps_fg, w_i, s, half):
                sl = slice(s * TS, (s + 1) * TS)
                base = 0 if half == 0 else KT
                rhs_t = xf8 if half == 0 else af8
                for q in range(NPR):
                    kk = slice(base + 2 * q, base + 2 * q + 2)
                    rr = slice(2 * q, 2 * q + 2)
                    nc.tensor.matmul(ps_ig, lhsT=w_i[:, kk, 0:P],
                                     rhs=rhs_t[:, rr, sl],
                                     start=(half == 0 and q == 0),
                                     stop=(half == 1 and q == NPR - 1),
                                     perf_mode=DR)
                for q in range(NPR):
                    kk = slice(base + 2 * q, base + 2 * q + 2)
                    rr = slice(2 * q, 2 * q + 2)
                    nc.tensor.matmul(ps_fg, lhsT=w_i[:, kk, P:2 * P],
                                     rhs=rhs_t[:, rr, sl],
                                     start=(half == 0 and q == 0),
                                     stop=(half == 1 and q == NPR - 1),
                                     perf_mode=DR)

            def epilogue(ps_ig, ps_fg, i, s, xe):
                """bf16 sigmoid outputs + all-bf16 combine: every DVE
                operand is 2-byte so the TT ops run in 2x_1p mode."""
                sl = slice(s * TS, (s + 1) * TS)
                sig_i = sigp.tile([P, TS], bf16, tag="sig")
                nc.scalar.activation(sig_i, ps_ig, SIG,
                                     bias=bias_sb[:, i:i + 1], scale=INV_S)
                sig_f = sigp.tile([P, TS], bf16, tag="sig")
                nc.scalar.activation(sig_f, ps_fg, SIG,
                                     bias=bias_sb[:, KT + i:KT + i + 1],
                                     scale=INV_S)
                nc.vector.tensor_mul(sig_i, sig_i, xe)
                nc.vector.tensor_mul(sig_f, sig_f, avgT_bf[:, i, sl])
                out_s = outp.tile([P, TS], bf16, tag="out")
                nc.vector.tensor_add(out_s, sig_i, sig_f)
                nc.gpsimd.dma_start(out=out_d[s, i], in_=out_s)

            def load_xe(i, s, eng):
                xe = xep.tile([P, TS], bf16, tag="xe")
                eng.dma_start(out=xe, in_=xe_d[s, i])
                return xe

            def full_unit(w_i, i, s, xe_eng):
                xe = load_xe(i, s, xe_eng)
                ps_ig = pp.tile([P, TS], f32, tag="ps")
                ps_fg = pp.tile([P, TS], f32, tag="ps")
                mm_halves(ps_ig, ps_fg, w_i, s, half=0)
                mm_halves(ps_ig, ps_fg, w_i, s, half=1)
                epilogue(ps_ig, ps_fg, i, s, xe)

            # ---- pass 1 (s = 0 across all i) ----
            run_xe = [load_xe(i, 0, nc.scalar) for i in range(RUNWAY)]
            # runway W avg-rows ride the gpsimd SWDGE queue, woven between
            # the slice-0 xd issues: the sync queue's serial transfer chain
            # (w0/xf8/w1x/w2x) is the startup bottleneck and these 1.5MB
            # would otherwise trail it by ~10us
            run_w = []
            for _ri in range(RUNWAY):
                w_tile = wpool.tile([P, NK, 2 * P], f8, tag="w")
                run_w.append(w_tile)
            scan_set0(inject={
                0: lambda: load_w_avg(run_w[0], 0, nc.gpsimd),
                5: lambda: load_w_avg(run_w[1], 1, nc.gpsimd),
                9: lambda: load_w_avg(run_w[2], 2, nc.gpsimd),
            })
            run_ps = []
            for i in range(RUNWAY):
                w_i = run_w[i]
                if i == 0:
                    # few, large transfers in strict need-order: each queue
                    # transfer carries ~1us fixed latency, so the first
                    # matmul's inputs (w0 pair-0 rows + xf8 j0-7) lead.
                    nc.sync.dma_start(out=w_i[:, 0:2, :],
                                      in_=w_src(0)[:, 0:2, :])
                    nc.sync.dma_start(out=xf8[:, 0:8, 0:TS],
                                      in_=xf8_d[:, 0, 0:8, :])
                    nc.sync.dma_start(out=w_i[:, 2:KT, :],
                                      in_=w_src(0)[:, 2:KT, :])
                    nc.sync.dma_start(out=xf8[:, 8:KT, 0:TS],
                                      in_=xf8_d[:, 0, 8:KT, :])
                else:
                    nc.sync.dma_start(out=w_i[:, 0:KT, :],
                                      in_=w_src(i)[:, 0:KT, :])
                ps_ig = pp.tile([P, TS], f32, tag="ps")
                ps_fg = pp.tile([P, TS], f32, tag="ps")
                mm_halves(ps_ig, ps_fg, w_i, 0, half=0)
                run_ps.append((w_i, ps_ig, ps_fg))
            for i in range(RUNWAY):
                w_i, ps_ig, ps_fg = run_ps[i]
                mm_halves(ps_ig, ps_fg, w_i, 0, half=1)
                epilogue(ps_ig, ps_fg, i, 0, run_xe[i])

            w_cur = load_w(RUNWAY)
            p2w0 = None
            for i in range(RUNWAY, DT):
                if i + 1 < DT:
                    w_nxt = load_w(i + 1)
                else:
                    w_nxt = p2w0        # pass-2 w0, prefetched at i==13
                issue_xd(5 if i >= 12 else 4)
                emit_scans(4 if i >= 13 else 3)
                full_unit(w_cur, i, 0, nc.scalar)
                if i == 5:
                    load_xf8_slice(1)
                elif i == 8:
                    load_xf8_slice(2)
                elif i == 10:
                    load_xf8_slice(3)
                elif i == 14:
                    p2w0 = load_w(0)   # after w15's load: don't cut its line
                w_cur = w_nxt

            # ---- pass 2: remaining slices, i-outer, W prefetched one
            # ---- i-group ahead; leftover scans finish in the first
            # ---- pass-2 units (their vector stream is nearly idle, but
            # ---- skip the very first unit so its sigmoids aren't queued
            # ---- behind scan-gated casts on the scalar stream) ----
            first_p2 = True
            for i in range(DT):
                w_i = w_cur
                for s in range(1, NS):
                    xe = load_xe(i, s, nc.gpsimd)
                    if not first_p2 and state["scan"] < len(pend):
                        issue_xd(3)
                        emit_scans(3)
                    first_p2 = False
                    if s == 2 and i + 1 < DT:
                        w_cur = load_w(i + 1)
                    ps_ig = pp.tile([P, TS], f32, tag="ps")
                    ps_fg = pp.tile([P, TS], f32, tag="ps")
                    mm_halves(ps_ig, ps_fg, w_i, s, half=0)
                    mm_halves(ps_ig, ps_fg, w_i, s, half=1)
                    epilogue(ps_ig, ps_fg, i, s, xe)

    nc.compile()
    return nc


def _get_compiled():
    global _compiled
    if _compiled is None:
        _compiled = _build()
    return _compiled


def _run(inputs, trace=False, **spmd_kwargs):
    from concourse.bass_utils import run_bass_kernel_spmd

    nc = _get_compiled()
    layer_in = np.asarray(inputs["layer_in"], dtype=np.float32)
    W_gate = np.asarray(inputs["W_gate"], dtype=np.float32)
    b_gate = np.asarray(inputs["b_gate"], dtype=np.float32)
    f8 = ml_dtypes.float8_e4m3
    bf = ml_dtypes.bfloat16

    # wP[i, p, kt*256+c]: W^T row kt*128+p, col (c>=128)*D + i*128 + (c%128),
    # x-rows (kt<16) scaled by S_W1, avg-rows by S_W2, fp8e4 clipped +-240.
    wT = np.ascontiguousarray(W_gate.T)                    # [k, o]
    w4 = wT.reshape(NK, P, 2, DT, P)
    scale = np.where(np.arange(NK) < KT, S_W1, S_W2).astype(np.float32)
    w5 = w4.transpose(3, 1, 0, 2, 4) * scale[None, None, :, None, None]
    wP = np.ascontiguousarray(
        np.clip(w5, -240, 240).astype(f8).reshape(DT, P, NK * 2 * P))

    tt = np.arange(T, dtype=np.float32)
    coef = (tt / (tt + 1.0)).reshape(1, T)
    inv = (1.0 / (tt + 1.0)).reshape(1, T)

    in_maps = []
    for bi in range(B):
        xT = np.ascontiguousarray(layer_in[bi].T)          # [D, T]
        xf8_h = (np.clip(xT * S_X, -240, 240).astype(f8)
                 .reshape(KT, P, NS, TS).transpose(1, 2, 0, 3))
        xe_h = (xT.astype(bf)
                .reshape(DT, P, NS, TS).transpose(2, 0, 1, 3))
        xd_h = ((xT * inv).astype(bf)
                .reshape(KT, P, NS, TS).transpose(2, 0, 1, 3))
        in_maps.append({
            "xf8T": np.ascontiguousarray(xf8_h),
            "xeT": np.ascontiguousarray(xe_h),
            "xdT": np.ascontiguousarray(xd_h),
            "wP": wP,
            "bias": b_gate,
            "coef_t": coef,
        })

    res = run_bass_kernel_spmd(nc, in_maps, core_ids=list(range(B)),
                               trace=trace, **spmd_kwargs)
    gating = np.empty((B, T, D), dtype=np.float32)
    avg = np.empty((B, T, D), dtype=np.float32)
    for bi in range(B):
        o4 = res.results[bi]["outT"].astype(np.float32)    # [NS, DT, P, TS]
        a4 = res.results[bi]["avgT"].astype(np.float32)
        gating[bi] = o4.transpose(1, 2, 0, 3).reshape(D, T).T
        avg[bi] = a4.transpose(1, 2, 0, 3).reshape(D, T).T
    return (gating, avg), res


def kernel(**inputs):
    (gating, avg), _ = _run(inputs, trace=False)
    return gating, avg
